# revision 1
# baseline (speedup 1.0000x reference)
"""GP posterior mean: mu = K_rbf(X_test, X_train) @ alpha on 8 NeuronCores.

Math: K[j,i] = sf2 * exp(-0.5*||xt_i - x_j||^2 / ell2).  The whole exponent is
expressed as a single dot product  exponent[j,i] = A[:,j] . B[:,i]  with a
14-long contraction built from bf16 hi/lo splits of the fp32 operands, so the
TensorE runs the distance matrix at full bf16 speed with ~fp32 accuracy.
The contraction is zero-padded to 128: sub-128 contractions keep the PE at
the throttled 1.2 GHz clock (only quadrant 0 active), while K=128 streams at
2.4 GHz.  ScalarE applies exp (sf2 folded into the activation bias), and a
second TensorE matmul contracts K against bf16 hi/lo-split alpha,
accumulating in PSUM over all train tiles.  Data-parallel over X_test rows:
each of the 8 cores handles 2048 test points with no communication.
"""

import numpy as np
import ml_dtypes

M = 16384
N = 16384
NCORES = 8
MC = M // NCORES          # 2048 test points per core
CHUNK = 1024              # test-chunk per ACT instruction (2 PSUM banks)
NCH = MC // CHUNK         # 2 chunks per core
NJT = N // 128            # 128 train tiles
C = 14                    # used contraction rows of the exponent matmul
CP = 128                  # padded contraction (keeps PE at full clock)

_cache = {}


def _split2(v):
    hi = v.astype(ml_dtypes.bfloat16)
    lo = (v - hi.astype(np.float64)).astype(ml_dtypes.bfloat16)
    return hi, lo


def _split3(v):
    hi = v.astype(ml_dtypes.bfloat16)
    r = v - hi.astype(np.float64)
    mid = r.astype(ml_dtypes.bfloat16)
    lo = (r - mid.astype(np.float64)).astype(ml_dtypes.bfloat16)
    return hi, mid, lo


def _build_program(bias):
    import concourse.mybir as mybir
    import concourse.tile as tile
    from concourse import bacc

    fp32 = mybir.dt.float32
    bf16 = mybir.dt.bfloat16

    nc = bacc.Bacc(None, target_bir_lowering=False)
    A_d = nc.declare_dram_parameter("A", [CP, N], bf16, isOutput=False)
    B_d = nc.declare_dram_parameter("B", [CP, MC], bf16, isOutput=False)
    AL_d = nc.declare_dram_parameter("AL", [128, NJT * 4], bf16, isOutput=False)
    OUT_d = nc.declare_dram_parameter("out", [4, MC], fp32, isOutput=True)

    with tile.TileContext(nc) as tc:
        with (
            tc.tile_pool(name="singles", bufs=1) as singles,
            tc.tile_pool(name="kpool", bufs=6) as kpool,
            tc.tile_pool(name="opool", bufs=2) as opool,
            tc.tile_pool(name="pse", bufs=3, space="PSUM") as pse,
            tc.tile_pool(name="psacc", bufs=1, space="PSUM") as psacc,
        ):
            sb_B = singles.tile([CP, MC], bf16)
            nc.sync.dma_start(out=sb_B, in_=B_d[:])
            sb_AL = singles.tile([128, NJT * 4], bf16)
            nc.gpsimd.dma_start(out=sb_AL, in_=AL_d[:])
            sb_A = singles.tile([CP, N], bf16)
            for ch in range(32):
                s = slice(ch * (N // 32), (ch + 1) * (N // 32))
                eng = nc.sync if ch % 2 == 0 else nc.gpsimd
                eng.dma_start(out=sb_A[:, s], in_=A_d[:, s])
            acc_all = psacc.tile([36, CHUNK], fp32, name="acc_all")
            accs = [acc_all[32 * i : 32 * i + 4, :] for i in range(NCH)]
            for jt in range(NJT):
                for c in range(NCH):
                    e = pse.tile([128, CHUNK], fp32)
                    for h in range(CHUNK // 512):
                        nc.tensor.matmul(
                            e[:, h * 512 : (h + 1) * 512],
                            lhsT=sb_A[:, jt * 128 : (jt + 1) * 128],
                            rhs=sb_B[
                                :, c * CHUNK + h * 512 : c * CHUNK + (h + 1) * 512
                            ],
                            start=True,
                            stop=True,
                        )
                    k = kpool.tile([128, CHUNK], bf16)
                    nc.scalar.activation(
                        k, e, mybir.ActivationFunctionType.Exp, bias=float(bias)
                    )
                    for h in range(CHUNK // 512):
                        nc.tensor.matmul(
                            accs[c][:, h * 512 : (h + 1) * 512],
                            lhsT=sb_AL[:, jt * 4 : (jt + 1) * 4],
                            rhs=k[:, h * 512 : (h + 1) * 512],
                            start=(jt == 0),
                            stop=(jt == NJT - 1),
                        )
            for c in range(NCH):
                o = opool.tile([4, CHUNK], fp32, name=f"o{c}")
                nc.vector.tensor_copy(o, accs[c])
                nc.sync.dma_start(
                    out=OUT_d[:, c * CHUNK : (c + 1) * CHUNK], in_=o
                )
    nc.compile()
    return nc


def _prep_inputs(X_test, X_train, alpha, log_lengthscale, log_outputscale):
    ell = np.exp(np.float32(log_lengthscale))
    ell2 = np.float64(np.float32(ell) ** 2)
    sf = np.exp(np.float32(log_outputscale))
    sf2 = np.float64(np.float32(sf) ** 2)

    xt = X_train.astype(np.float64)
    xs = X_test.astype(np.float64)
    al = alpha.astype(np.float64)

    # Train-side matrix A (CP, N); rows 14.. are zero padding
    x0h, x0l = _split2(xt[:, 0])
    x1h, x1l = _split2(xt[:, 1])
    pj = -(xt[:, 0] ** 2 + xt[:, 1] ** 2) / (2.0 * ell2)
    pjh, pjm, pjl = _split3(pj)
    ones = np.ones(N, dtype=ml_dtypes.bfloat16)
    A = np.zeros((CP, N), dtype=ml_dtypes.bfloat16)
    A[:C] = np.stack(
        [ones, ones, ones, x0h, x0h, x0l, x0l, x1h, x1h, x1l, x1l, pjh, pjm, pjl]
    )

    # Test-side matrix B (CP, M); rows 14.. are zero padding
    T0 = -(xs[:, 0] ** 2 + xs[:, 1] ** 2) / (2.0 * ell2)
    T0h, T0m, T0l = _split3(T0)
    u0 = xs[:, 0] / ell2
    u0h, u0l = _split2(u0)
    u1 = xs[:, 1] / ell2
    u1h, u1l = _split2(u1)
    onesM = np.ones(M, dtype=ml_dtypes.bfloat16)
    B = np.zeros((CP, M), dtype=ml_dtypes.bfloat16)
    B[:C] = np.stack(
        [T0h, T0m, T0l, u0h, u0l, u0h, u0l, u1h, u1l, u1h, u1l, onesM, onesM, onesM]
    )

    # alpha tiles (128, NJT*4): hi/lo split of each alpha column
    arh, arl = _split2(al[:, 0])
    aih, ail = _split2(al[:, 1])
    AL = np.stack([arh, arl, aih, ail], axis=1)  # (N, 4)
    AL = AL.reshape(NJT, 128, 4).transpose(1, 0, 2).reshape(128, NJT * 4)
    AL = np.ascontiguousarray(AL)

    bias = np.float32(np.log(sf2))
    return A, B, AL, bias


def kernel(X_test, X_train, alpha, log_lengthscale, log_outputscale):
    from concourse.bass_utils import run_bass_kernel_spmd

    A, B, AL, bias = _prep_inputs(
        X_test, X_train, alpha, log_lengthscale, log_outputscale
    )

    key = ("nc", float(bias))
    if key not in _cache:
        _cache[key] = _build_program(bias)
    nc = _cache[key]

    core_ids = list(range(NCORES))
    in_maps = []
    for c in core_ids:
        in_maps.append(
            {
                "A": A,
                "B": np.ascontiguousarray(B[:, c * MC : (c + 1) * MC]),
                "AL": AL,
            }
        )
    res = run_bass_kernel_spmd(nc, in_maps, core_ids)

    out = np.empty((M, 2), dtype=np.float32)
    for c in core_ids:
        o = res.results[c]["out"]
        out[c * MC : (c + 1) * MC, 0] = o[0] + o[1]
        out[c * MC : (c + 1) * MC, 1] = o[2] + o[3]
    return out



# revision 4
# speedup vs baseline: 3.9906x; 3.9906x over previous
"""GP posterior mean: mu = K_rbf(X_test, X_train) @ alpha on 8 NeuronCores.

Sparse neighbor-window formulation.  With ell=0.1 the RBF kernel is narrow:
pairs beyond r = sqrt(2*ell^2*T) (T=8 -> r=0.4) contribute < 1e-4 relative
error, so only ~1.4% of the 16384^2 pairs need computing.  The host splits
X_test into compact 128-point chunks (KD bisection), finds each chunk's exact
union of train neighbors within r, and packs them into 128-point train
"slots" (padded with clamped indices whose alpha is zeroed).  Chunks are
dealt round-robin by descending slot count so every core receives an
identical slot structure -> one SPMD program, per-core data via DMA.

On device, per slot: a K=14 exponent matmul (bf16 hi/lo split contraction,
as in the dense kernel) writes a [128, 128] tile into a PSUM group; one
ScalarE Exp activation per 1536-column group (batched to amortize the
~352-cycle ACT instruction overhead that made the dense kernel ACT-bound);
then a [128,4] alpha matmul accumulates K @ alpha into a per-chunk PSUM
accumulator.  ScalarE work drops from 256 ACTs x 1024 cols to ~16 ACTs x
1536 cols per core.
"""

import numpy as np
import ml_dtypes

M = 16384
N = 16384
NCORES = 8
CSIZE = 128               # test points per chunk
NCH = M // CSIZE // NCORES  # chunks per core (16)
C = 14                    # contraction rows of the exponent matmul
T_CUT = 8.0               # exponent cutoff: drop pairs with exponent < -T
GROUP = 12                # slots per exp-activation group (12*128 = 1536 cols)

_cache = {}


def _split2(v):
    hi = v.astype(ml_dtypes.bfloat16)
    lo = (v - hi.astype(np.float64)).astype(ml_dtypes.bfloat16)
    return hi, lo


def _split3(v):
    hi = v.astype(ml_dtypes.bfloat16)
    r = v - hi.astype(np.float64)
    mid = r.astype(ml_dtypes.bfloat16)
    lo = (r - mid.astype(np.float64)).astype(ml_dtypes.bfloat16)
    return hi, mid, lo


def _kd_chunks(X, csize):
    """Recursive bisection into spatially compact chunks of exactly csize."""
    out = []

    def rec(ids):
        if len(ids) <= csize:
            out.append(ids)
            return
        pts = X[ids]
        ext = pts.max(0) - pts.min(0)
        ax = int(ext[1] > ext[0])
        k = (len(ids) // 2 // csize) * csize
        if k == 0:
            k = len(ids) // 2
        o = np.argsort(pts[:, ax], kind="stable")
        rec(ids[o[:k]])
        rec(ids[o[k:]])

    rec(np.arange(len(X)))
    return out


def _neighbors(chunk_pts, X_train, xorder, xsorted, r):
    """Exact union of train indices within r of any chunk point."""
    xmin, ymin = chunk_pts.min(0) - r
    xmax, ymax = chunk_pts.max(0) + r
    lo = np.searchsorted(xsorted, xmin)
    hi = np.searchsorted(xsorted, xmax)
    cand = xorder[lo:hi]
    cp = X_train[cand]
    cand = cand[(cp[:, 1] >= ymin) & (cp[:, 1] <= ymax)]
    if len(cand) == 0:
        return cand
    cp = X_train[cand]
    d2 = ((cp[:, None, :] - chunk_pts[None, :, :]) ** 2).sum(-1).min(1)
    return cand[d2 <= r * r]


def _prep(X_test, X_train, alpha, log_lengthscale, log_outputscale):
    ell = np.exp(np.float32(log_lengthscale))
    ell2 = np.float64(np.float32(ell) ** 2)
    sf = np.exp(np.float32(log_outputscale))
    sf2 = np.float64(np.float32(sf) ** 2)
    bias = float(np.float32(np.log(sf2)))
    r = float(np.sqrt(2.0 * ell2 * T_CUT))

    xt = np.asarray(X_train, dtype=np.float64)
    xs = np.asarray(X_test, dtype=np.float64)
    al = np.asarray(alpha, dtype=np.float64)

    # ---- train-side split matrix A (14, N) and alpha splits (N, 4) ----
    x0h, x0l = _split2(xt[:, 0])
    x1h, x1l = _split2(xt[:, 1])
    pj = -(xt[:, 0] ** 2 + xt[:, 1] ** 2) / (2.0 * ell2)
    pjh, pjm, pjl = _split3(pj)
    ones = np.ones(N, dtype=ml_dtypes.bfloat16)
    A14 = np.stack(
        [ones, ones, ones, x0h, x0h, x0l, x0l, x1h, x1h, x1l, x1l, pjh, pjm, pjl]
    )

    arh, arl = _split2(al[:, 0])
    aih, ail = _split2(al[:, 1])
    ALfull = np.stack([arh, arl, aih, ail], axis=1)  # (N, 4)

    # ---- test-side split matrix B (14, M) ----
    T0 = -(xs[:, 0] ** 2 + xs[:, 1] ** 2) / (2.0 * ell2)
    T0h, T0m, T0l = _split3(T0)
    u0 = xs[:, 0] / ell2
    u0h, u0l = _split2(u0)
    u1 = xs[:, 1] / ell2
    u1h, u1l = _split2(u1)
    onesM = np.ones(M, dtype=ml_dtypes.bfloat16)
    B14 = np.stack(
        [T0h, T0m, T0l, u0h, u0l, u0h, u0l, u1h, u1l, u1h, u1l, onesM, onesM, onesM]
    )

    # ---- chunks + exact neighbor unions ----
    chunks = _kd_chunks(np.asarray(X_test, dtype=np.float64), CSIZE)
    xorder = np.argsort(xt[:, 0], kind="stable")
    xsorted = xt[xorder, 0]
    nbrs = []
    ntiles = np.empty(len(chunks), dtype=np.int64)
    for i, ids in enumerate(chunks):
        nb = _neighbors(xs[ids], xt, xorder, xsorted, r)
        nbrs.append(nb)
        ntiles[i] = max(1, -(-len(nb) // 128))

    # ---- deal chunks to cores: round-robin by descending tile count ----
    rank = np.argsort(-ntiles, kind="stable")
    NT = []  # tiles per round (same for every core)
    core_chunks = [[] for _ in range(NCORES)]
    for i in range(NCH):
        grp = rank[i * NCORES : (i + 1) * NCORES]
        NT.append(int(ntiles[grp].max()))
        for c in range(NCORES):
            core_chunks[c].append(grp[c])
    NT = tuple(NT)
    S = sum(NT)

    # ---- per-core gathered tensors ----
    in_maps = []
    perms = []
    for c in range(NCORES):
        test_idx = np.concatenate([chunks[ci] for ci in core_chunks[c]])
        perms.append(test_idx)
        B_work = np.ascontiguousarray(B14[:, test_idx])

        tr_idx = np.zeros(S * 128, dtype=np.int64)
        al_mask = np.zeros(S * 128, dtype=bool)
        pos = 0
        for i, ci in enumerate(core_chunks[c]):
            nb = nbrs[ci]
            tr_idx[pos : pos + len(nb)] = nb
            al_mask[pos : pos + len(nb)] = True
            pos += NT[i] * 128
        A_work = np.ascontiguousarray(A14[:, tr_idx])
        ALw = ALfull[tr_idx]
        ALw[~al_mask] = 0
        # slot s holds its 128 train points on partitions, 4 cols each
        AL_work = np.ascontiguousarray(
            ALw.reshape(S, 128, 4).transpose(1, 0, 2).reshape(128, S * 4)
        )
        in_maps.append({"A": A_work, "B": B_work, "AL": AL_work})

    meta = {"NT": NT, "S": S, "bias": bias, "perms": perms}
    return in_maps, meta


def _build_program(bias, NT):
    import concourse.mybir as mybir
    import concourse.tile as tile
    from concourse import bacc

    fp32 = mybir.dt.float32
    bf16 = mybir.dt.bfloat16

    S = sum(NT)
    MC = NCH * CSIZE  # test points per core

    # flat slot list: (chunk_round, slot_index_within_chunk)
    slots = []
    for i, nt in enumerate(NT):
        for j in range(nt):
            slots.append((i, j))

    nc = bacc.Bacc(None, target_bir_lowering=False)
    A_d = nc.declare_dram_parameter("A", [C, S * 128], bf16, isOutput=False)
    B_d = nc.declare_dram_parameter("B", [C, MC], bf16, isOutput=False)
    AL_d = nc.declare_dram_parameter("AL", [128, S * 4], bf16, isOutput=False)
    OUT_d = nc.declare_dram_parameter("out", [4, MC], fp32, isOutput=True)

    with tile.TileContext(nc) as tc:
        with (
            tc.tile_pool(name="singles", bufs=1) as singles,
            tc.tile_pool(name="kpool", bufs=3) as kpool,
            tc.tile_pool(name="opool", bufs=2) as opool,
            tc.tile_pool(name="pse", bufs=2, space="PSUM") as pse,
            tc.tile_pool(name="psacc", bufs=2, space="PSUM") as psacc,
        ):
            sb_B = singles.tile([C, MC], bf16)
            nc.sync.dma_start(out=sb_B, in_=B_d[:])
            sb_AL = singles.tile([128, S * 4], bf16)
            nc.gpsimd.dma_start(out=sb_AL, in_=AL_d[:])
            # A in 4 tiles so early slots don't wait on the full transfer
            NA = 4
            per = -(-S // NA)
            sb_As = []
            for t in range(NA):
                s0 = t * per
                s1 = min(S, (t + 1) * per)
                ta = singles.tile([C, (s1 - s0) * 128], bf16, name=f"A{t}")
                eng = nc.sync if t % 2 == 0 else nc.gpsimd
                eng.dma_start(out=ta, in_=A_d[:, s0 * 128 : s1 * 128])
                sb_As.append((s0, ta))

            def a_slot(s):
                t = min(s // per, NA - 1)
                s0, ta = sb_As[t]
                return ta[:, (s - s0) * 128 : (s - s0 + 1) * 128]

            ngroups = -(-S // GROUP)
            acc = None
            for g in range(ngroups):
                lo = g * GROUP
                hi = min(S, (g + 1) * GROUP)
                w = (hi - lo) * 128
                e = pse.tile([128, w], fp32)
                for j, s in enumerate(range(lo, hi)):
                    ch, _ = slots[s]
                    nc.tensor.matmul(
                        e[:, j * 128 : (j + 1) * 128],
                        lhsT=a_slot(s),
                        rhs=sb_B[:, ch * CSIZE : (ch + 1) * CSIZE],
                        start=True,
                        stop=True,
                    )
                k = kpool.tile([128, w], bf16)
                nc.scalar.activation(
                    k, e, mybir.ActivationFunctionType.Exp, bias=float(bias)
                )
                for j, s in enumerate(range(lo, hi)):
                    ch, idx = slots[s]
                    if ch % 4 == 0 and idx == 0:
                        acc = psacc.tile([4, 4 * CSIZE], fp32, name="acc")
                    nc.tensor.matmul(
                        acc[:, (ch % 4) * CSIZE : (ch % 4 + 1) * CSIZE],
                        lhsT=sb_AL[:, s * 4 : (s + 1) * 4],
                        rhs=k[:, j * 128 : (j + 1) * 128],
                        start=(idx == 0),
                        stop=(idx == NT[ch] - 1),
                    )
                    if idx == NT[ch] - 1 and ch % 4 == 3:
                        b = ch // 4
                        o = opool.tile([4, 4 * CSIZE], fp32, name="o")
                        nc.vector.tensor_copy(o, acc)
                        nc.sync.dma_start(
                            out=OUT_d[:, b * 4 * CSIZE : (b + 1) * 4 * CSIZE], in_=o
                        )
    nc.compile()
    return nc


def _assemble(res, meta, core_ids):
    out = np.empty((M, 2), dtype=np.float32)
    for c in core_ids:
        o = res.results[c]["out"]
        idx = meta["perms"][c]
        out[idx, 0] = o[0] + o[1]
        out[idx, 1] = o[2] + o[3]
    return out


def kernel(X_test, X_train, alpha, log_lengthscale, log_outputscale):
    from concourse.bass_utils import run_bass_kernel_spmd

    in_maps, meta = _prep(X_test, X_train, alpha, log_lengthscale, log_outputscale)

    key = (meta["bias"], meta["NT"])
    if key not in _cache:
        _cache.clear()
        _cache[key] = _build_program(meta["bias"], meta["NT"])
    nc = _cache[key]

    core_ids = list(range(NCORES))
    res = run_bass_kernel_spmd(nc, in_maps, core_ids)
    return _assemble(res, meta, core_ids)


# revision 7
# speedup vs baseline: 6.2346x; 1.5623x over previous
"""GP posterior mean: mu = K_rbf(X_test, X_train) @ alpha on 8 NeuronCores.

Sparse neighbor-window formulation.  With ell=0.1 the RBF kernel is narrow:
pairs beyond r = sqrt(2*ell^2*T) (T=8 -> r=0.4) contribute < 1e-4 relative
error, so only ~1.4% of the 16384^2 pairs need computing.  The host splits
X_test into compact 128-point chunks (KD bisection), finds each chunk's exact
union of train neighbors within r, and packs them into 128-point train
"slots" (padded with clamped indices whose alpha is zeroed).  Chunks are
dealt round-robin by descending slot count so every core receives an
identical slot structure -> one SPMD program, per-core data via DMA.

On device, per slot: an exponent matmul (bf16 hi/lo split contraction as in
the dense kernel, zero-padded to K=128 so the PE runs warm with fast weight
load) writes a [128, 128] tile into a PSUM group; one ScalarE Exp per
1536-column group (batched to amortize the ~352-cycle ACT overhead that made
the dense kernel ACT-bound); then a [128, 4] alpha matmul accumulates
K @ alpha into the chunk's PSUM accumulator.  The alpha matmuls of 4
consecutive slots are packed into the 4 PE column-groups (tile_position
auto-derived from the PSUM base partition) so they run concurrently.
"""

import numpy as np
import ml_dtypes

M = 16384
N = 16384
NCORES = 8
CSIZE = 128                 # test points per chunk
NCH = M // CSIZE // NCORES  # chunks per core (16)
C = 14                      # live contraction rows of the exponent matmul
CP = 128                    # padded contraction (warm PE clock + FWL)
T_CUT = 8.0                 # exponent cutoff: drop pairs with exponent < -T
GROUP = 12                  # slots per exp-activation group (12*128 = 1536 cols)
NA = 8                      # A transfers (DMA/compute overlap)

_cache = {}


def _split2(v):
    hi = v.astype(ml_dtypes.bfloat16)
    lo = (v - hi.astype(np.float64)).astype(ml_dtypes.bfloat16)
    return hi, lo


def _split3(v):
    hi = v.astype(ml_dtypes.bfloat16)
    r = v - hi.astype(np.float64)
    mid = r.astype(ml_dtypes.bfloat16)
    lo = (r - mid.astype(np.float64)).astype(ml_dtypes.bfloat16)
    return hi, mid, lo


def _kd_chunks(X, csize):
    """Recursive bisection into spatially compact chunks of exactly csize."""
    out = []

    def rec(ids):
        if len(ids) <= csize:
            out.append(ids)
            return
        pts = X[ids]
        ext = pts.max(0) - pts.min(0)
        ax = int(ext[1] > ext[0])
        k = (len(ids) // 2 // csize) * csize
        if k == 0:
            k = len(ids) // 2
        o = np.argsort(pts[:, ax], kind="stable")
        rec(ids[o[:k]])
        rec(ids[o[k:]])

    rec(np.arange(len(X)))
    return out


def _neighbors(chunk_pts, X_train, xorder, xsorted, r):
    """Exact union of train indices within r of any chunk point."""
    xmin, ymin = chunk_pts.min(0) - r
    xmax, ymax = chunk_pts.max(0) + r
    lo = np.searchsorted(xsorted, xmin)
    hi = np.searchsorted(xsorted, xmax)
    cand = xorder[lo:hi]
    cp = X_train[cand]
    cand = cand[(cp[:, 1] >= ymin) & (cp[:, 1] <= ymax)]
    if len(cand) == 0:
        return cand
    cp = X_train[cand]
    d2 = ((cp[:, None, :] - chunk_pts[None, :, :]) ** 2).sum(-1).min(1)
    return cand[d2 <= r * r]


def _prep(X_test, X_train, alpha, log_lengthscale, log_outputscale):
    ell = np.exp(np.float32(log_lengthscale))
    ell2 = np.float64(np.float32(ell) ** 2)
    sf = np.exp(np.float32(log_outputscale))
    sf2 = np.float64(np.float32(sf) ** 2)
    bias = float(np.float32(np.log(sf2)))
    r = float(np.sqrt(2.0 * ell2 * T_CUT))

    xt = np.asarray(X_train, dtype=np.float64)
    xs = np.asarray(X_test, dtype=np.float64)
    al = np.asarray(alpha, dtype=np.float64)

    # ---- train-side split matrix A (CP, N) and alpha splits (N, 4) ----
    x0h, x0l = _split2(xt[:, 0])
    x1h, x1l = _split2(xt[:, 1])
    pj = -(xt[:, 0] ** 2 + xt[:, 1] ** 2) / (2.0 * ell2)
    pjh, pjm, pjl = _split3(pj)
    ones = np.ones(N, dtype=ml_dtypes.bfloat16)
    A14 = np.zeros((CP, N), dtype=ml_dtypes.bfloat16)
    A14[:C] = np.stack(
        [ones, ones, ones, x0h, x0h, x0l, x0l, x1h, x1h, x1l, x1l, pjh, pjm, pjl]
    )

    arh, arl = _split2(al[:, 0])
    aih, ail = _split2(al[:, 1])
    ALfull = np.stack([arh, arl, aih, ail], axis=1)  # (N, 4)

    # ---- test-side split matrix B (CP, M) ----
    T0 = -(xs[:, 0] ** 2 + xs[:, 1] ** 2) / (2.0 * ell2)
    T0h, T0m, T0l = _split3(T0)
    u0 = xs[:, 0] / ell2
    u0h, u0l = _split2(u0)
    u1 = xs[:, 1] / ell2
    u1h, u1l = _split2(u1)
    onesM = np.ones(M, dtype=ml_dtypes.bfloat16)
    B14 = np.zeros((CP, M), dtype=ml_dtypes.bfloat16)
    B14[:C] = np.stack(
        [T0h, T0m, T0l, u0h, u0l, u0h, u0l, u1h, u1l, u1h, u1l, onesM, onesM, onesM]
    )

    # ---- chunks + exact neighbor unions ----
    chunks = _kd_chunks(np.asarray(X_test, dtype=np.float64), CSIZE)
    xorder = np.argsort(xt[:, 0], kind="stable")
    xsorted = xt[xorder, 0]
    nbrs = []
    ntiles = np.empty(len(chunks), dtype=np.int64)
    for i, ids in enumerate(chunks):
        nb = _neighbors(xs[ids], xt, xorder, xsorted, r)
        nbrs.append(nb)
        ntiles[i] = max(1, -(-len(nb) // 128))

    # ---- deal chunks to cores: round-robin by descending tile count ----
    rank = np.argsort(-ntiles, kind="stable")
    NT = []  # tiles per round (same for every core)
    core_chunks = [[] for _ in range(NCORES)]
    for i in range(NCH):
        grp = rank[i * NCORES : (i + 1) * NCORES]
        NT.append(int(ntiles[grp].max()))
        for c in range(NCORES):
            core_chunks[c].append(grp[c])
    NT = tuple(NT)
    S = sum(NT)

    # ---- per-core gathered tensors ----
    in_maps = []
    perms = []
    for c in range(NCORES):
        test_idx = np.concatenate([chunks[ci] for ci in core_chunks[c]])
        perms.append(test_idx)
        B_work = np.ascontiguousarray(B14[:, test_idx])

        tr_idx = np.zeros(S * 128, dtype=np.int64)
        al_mask = np.zeros(S * 128, dtype=bool)
        pos = 0
        for i, ci in enumerate(core_chunks[c]):
            nb = nbrs[ci]
            tr_idx[pos : pos + len(nb)] = nb
            al_mask[pos : pos + len(nb)] = True
            pos += NT[i] * 128
        A_work = np.ascontiguousarray(A14[:, tr_idx])
        ALw = ALfull[tr_idx]
        ALw[~al_mask] = 0
        # slot s holds its 128 train points on partitions, 4 cols each
        AL_work = np.ascontiguousarray(
            ALw.reshape(S, 128, 4).transpose(1, 0, 2).reshape(128, S * 4)
        )
        in_maps.append({"A": A_work, "B": B_work, "AL": AL_work})

    meta = {"NT": NT, "S": S, "bias": bias, "perms": perms}
    return in_maps, meta


def _build_program(bias, NT):
    import concourse.mybir as mybir
    import concourse.tile as tile
    from concourse import bacc

    fp32 = mybir.dt.float32
    bf16 = mybir.dt.bfloat16

    S = sum(NT)
    MC = NCH * CSIZE  # test points per core

    # flat slot list: (chunk_round, slot_index_within_chunk)
    slots = []
    for i, nt in enumerate(NT):
        for j in range(nt):
            slots.append((i, j))

    nc = bacc.Bacc(None, target_bir_lowering=False)
    A_d = nc.declare_dram_parameter("A", [CP, S * 128], bf16, isOutput=False)
    B_d = nc.declare_dram_parameter("B", [CP, MC], bf16, isOutput=False)
    AL_d = nc.declare_dram_parameter("AL", [128, S * 4], bf16, isOutput=False)
    OUT_d = nc.declare_dram_parameter("out", [12, MC], fp32, isOutput=True)

    with tile.TileContext(nc) as tc:
        with (
            tc.tile_pool(name="singles", bufs=1) as singles,
            tc.tile_pool(name="kpool", bufs=3) as kpool,
            tc.tile_pool(name="opool", bufs=2) as opool,
            tc.tile_pool(name="pse", bufs=2, space="PSUM") as pse,
            tc.tile_pool(name="psacc", bufs=2, space="PSUM") as psacc,
        ):
            sb_B = singles.tile([CP, MC], bf16)
            nc.sync.dma_start(out=sb_B, in_=B_d[:])
            sb_AL = singles.tile([128, S * 4], bf16)
            nc.gpsimd.dma_start(out=sb_AL, in_=AL_d[:])
            # A in NA tiles, consumption-ordered, so compute overlaps the DMA
            per = -(-S // NA)
            sb_As = []
            for t in range(NA):
                s0 = t * per
                s1 = min(S, (t + 1) * per)
                ta = singles.tile([CP, (s1 - s0) * 128], bf16, name=f"A{t}")
                eng = nc.sync if t % 2 == 0 else nc.gpsimd
                eng.dma_start(out=ta, in_=A_d[:, s0 * 128 : s1 * 128])
                sb_As.append((s0, ta))

            def a_slot(s):
                t = min(s // per, NA - 1)
                s0, ta = sb_As[t]
                return ta[:, (s - s0) * 128 : (s - s0 + 1) * 128]

            ngroups = -(-S // GROUP)
            acc = None
            for g in range(ngroups):
                lo = g * GROUP
                hi = min(S, (g + 1) * GROUP)
                w = (hi - lo) * 128
                e = pse.tile([128, w], fp32)
                for j, s in enumerate(range(lo, hi)):
                    ch, _ = slots[s]
                    nc.tensor.matmul(
                        e[:, j * 128 : (j + 1) * 128],
                        lhsT=a_slot(s),
                        rhs=sb_B[:, ch * CSIZE : (ch + 1) * CSIZE],
                        start=True,
                        stop=True,
                    )
                k = kpool.tile([128, w], bf16)
                nc.scalar.activation(
                    k, e, mybir.ActivationFunctionType.Exp, bias=float(bias)
                )
                for j, s in enumerate(range(lo, hi)):
                    ch, idx = slots[s]
                    if ch % 4 == 0 and idx == 0:
                        acc = psacc.tile([68, 4 * CSIZE], fp32, name="acc")
                    cg = idx % 3  # PE column-group: concurrent alpha matmuls
                    nc.tensor.matmul(
                        acc[
                            32 * cg : 32 * cg + 4,
                            (ch % 4) * CSIZE : (ch % 4 + 1) * CSIZE,
                        ],
                        lhsT=sb_AL[:, s * 4 : (s + 1) * 4],
                        rhs=k[:, j * 128 : (j + 1) * 128],
                        start=(idx == cg),
                        stop=(idx >= NT[ch] - 3),
                    )
                    if idx == NT[ch] - 1 and ch % 4 == 3:
                        b = ch // 4
                        o = opool.tile([68, 4 * CSIZE], fp32, name="o")
                        nc.vector.tensor_copy(o, acc)
                        for cg2 in range(3):
                            nc.sync.dma_start(
                                out=OUT_d[
                                    4 * cg2 : 4 * cg2 + 4,
                                    b * 4 * CSIZE : (b + 1) * 4 * CSIZE,
                                ],
                                in_=o[32 * cg2 : 32 * cg2 + 4, :],
                            )
    nc.compile()
    return nc


def _assemble(res, meta, core_ids):
    NT = meta["NT"]
    out = np.empty((M, 2), dtype=np.float32)
    for c in core_ids:
        o = res.results[c]["out"].astype(np.float64)  # (16, MC)
        idx = meta["perms"][c]
        re = np.zeros(NCH * CSIZE)
        im = np.zeros(NCH * CSIZE)
        for i in range(NCH):
            sl = slice(i * CSIZE, (i + 1) * CSIZE)
            for cg in range(min(3, NT[i])):
                re[sl] += o[4 * cg + 0, sl] + o[4 * cg + 1, sl]
                im[sl] += o[4 * cg + 2, sl] + o[4 * cg + 3, sl]
        out[idx, 0] = re
        out[idx, 1] = im
    return out


def kernel(X_test, X_train, alpha, log_lengthscale, log_outputscale):
    from concourse.bass_utils import run_bass_kernel_spmd

    in_maps, meta = _prep(X_test, X_train, alpha, log_lengthscale, log_outputscale)

    key = (meta["bias"], meta["NT"])
    if key not in _cache:
        _cache.clear()
        _cache[key] = _build_program(meta["bias"], meta["NT"])
    nc = _cache[key]

    core_ids = list(range(NCORES))
    res = run_bass_kernel_spmd(nc, in_maps, core_ids)
    return _assemble(res, meta, core_ids)
